# revision 50
# baseline (speedup 1.0000x reference)
"""Trainium2 Bass kernel for nn_CollaborativeExpertsWrapper.

Self-contained: shards batch B=128 across 8 NeuronCores (data-parallel
encoders), all-gathers [16, 2048] embeddings, each core redundantly computes
the masked ranking loss; host takes core 0's (loss, acc).

v2: full-bf16 datapath — inputs and weights are cast to bf16 on the host
(halves HBM traffic), xT is produced by HWDGE transpose-DMA (removes the
PE-transpose + ACT-copy pipeline), all projection matmuls run in bf16.
Accumulation stays fp32 in PSUM; the ranking block stays fp32.
"""
import sys

sys.path.insert(0, "/opt/trn_rl_repo")

import math
from contextlib import ExitStack

import numpy as np

import concourse.bacc as bacc
import concourse.bass as bass
import concourse.mybir as mybir
import concourse.tile as tile
from concourse.alu_op_type import AluOpType
from concourse.masks import make_identity

F32 = mybir.dt.float32
F32R = mybir.dt.float32r
BF16 = mybir.dt.bfloat16
U8 = mybir.dt.uint8
AF = mybir.ActivationFunctionType
AX = mybir.AxisListType

N_CORES = 8
B = 128
BL = B // N_CORES  # 16 samples per core
T = 64
DIM = 512
HEADS = 4
HD = DIM // HEADS  # 128
MARGIN = 1.0
TOK = BL * T  # 1024 tokens per core per modality
O_T = 1024
ODIM = 512

_CACHE = {}


def _build():
    nc = bacc.Bacc("TRN2", target_bir_lowering=False, debug=False, num_devices=N_CORES)

    o_d = nc.dram_tensor("o", [BL, O_T, ODIM], BF16, kind="ExternalInput").ap()
    rgb_d = nc.dram_tensor("rgb", [BL, T, 2048], BF16, kind="ExternalInput").ap()
    aud_d = nc.dram_tensor("audio", [BL, T, 128], BF16, kind="ExternalInput").ap()
    gm_d = nc.dram_tensor("group_mask", [B], U8, kind="ExternalInput").ap()

    wd = {}
    for m, dm in (("rgb", 2048), ("audio", 128)):
        for p in "qkv":
            wd[f"{m}_W{p}"] = nc.dram_tensor(f"{m}_W{p}", [dm, DIM], BF16, kind="ExternalInput").ap()
            wd[f"{m}_b{p}"] = nc.dram_tensor(f"{m}_b{p}", [DIM], F32, kind="ExternalInput").ap()
        wd[f"{m}_Wo"] = nc.dram_tensor(f"{m}_Wo", [DIM, DIM], BF16, kind="ExternalInput").ap()
        wd[f"{m}_bo"] = nc.dram_tensor(f"{m}_bo", [DIM], F32, kind="ExternalInput").ap()
        wd[f"{m}_W2"] = nc.dram_tensor(f"{m}_W2", [DIM, DIM], BF16, kind="ExternalInput").ap()
        wd[f"{m}_b2"] = nc.dram_tensor(f"{m}_b2", [DIM], F32, kind="ExternalInput").ap()
    wd["expand_W"] = nc.dram_tensor("expand_W", [DIM, 2 * DIM], BF16, kind="ExternalInput").ap()
    wd["expand_b"] = nc.dram_tensor("expand_b", [2 * DIM], F32, kind="ExternalInput").ap()

    out_d = nc.dram_tensor("out", [1, 6], F32, kind="ExternalOutput").ap()

    import os
    stage = os.environ.get("KSTAGE", "full")
    dbg_d = None
    if stage != "full":
        dbg_d = nc.dram_tensor("dbg", [B, 4 * DIM], F32, kind="ExternalOutput").ap()

    with tile.TileContext(nc) as tc:
        _emit(nc, tc, o_d, rgb_d, aud_d, gm_d, wd, out_d, stage, dbg_d)

    nc.compile()
    return nc


def _emit(nc, tc, o_d, rgb_d, aud_d, gm_d, wd, out_d, stage="full", dbg_d=None):
    stk = ExitStack()
    with stk:
        const = stk.enter_context(tc.tile_pool(name="const", bufs=1))
        persist = stk.enter_context(tc.tile_pool(name="persist", bufs=1))
        ps = stk.enter_context(tc.tile_pool(name="psum", bufs=7, space="PSUM"))
        dram = stk.enter_context(tc.tile_pool(name="dram", bufs=1, space="DRAM"))

        def pst(shape, tag="ps", bufs=None):
            return ps.tile(shape, F32, tag=tag, bufs=bufs, name=tag)

        # ---------------- constants ----------------
        ident = const.tile([128, 128], F32, tag="ident")
        make_identity(nc, ident)
        ones_col_f32 = const.tile([128, 1], F32, tag="ones_col_f32")
        nc.vector.memset(ones_col_f32[:], 1.0)
        ones64_s = const.tile([128, 128], F32, tag="ones64_s")
        nc.vector.memset(ones64_s[:], 0.0)
        nc.vector.memset(ones64_s[0:64, 0:64], 1.0)
        nc.vector.memset(ones64_s[64:128, 64:128], 1.0)
        ones_row_f32 = const.tile([1, 128], F32, tag="ones_row_f32")
        nc.vector.memset(ones_row_f32[:], 1.0)
        ones128 = const.tile([128, 128], F32, tag="ones128")
        nc.vector.memset(ones128[:], 1.0)
        ones_row_bf = const.tile([1, 128], BF16, tag="ones_row_bf")
        nc.vector.tensor_copy(ones_row_bf[:], ones_row_f32[:])
        sel16_s = const.tile([128, BL, BL], F32, tag="sel16_s")
        nc.vector.memset(sel16_s[:], 0.0)
        for b in range(BL):
            nc.vector.memset(sel16_s[:, b, b : b + 1], 1.0)
        sel16 = const.tile([128, BL, BL], BF16, tag="sel16")
        nc.vector.tensor_copy(sel16[:], sel16_s[:])
        ones128_bf = const.tile([128, 128], BF16, tag="ones128_bf")
        nc.vector.tensor_copy(ones128_bf[:], ones128[:])

        g_row_u8 = const.tile([1, B], U8, tag="g_row_u8")
        nc.sync.dma_start(g_row_u8[:], gm_d[None, :])
        g_row = const.tile([1, B], F32, tag="g_row")
        nc.vector.tensor_copy(g_row[:], g_row_u8[:])
        g_col_u8 = const.tile([B, 1], U8, tag="g_col_u8")
        nc.sync.dma_start(g_col_u8[:], gm_d[:, None])
        g_col = const.tile([B, 1], F32, tag="g_col")
        nc.vector.tensor_copy(g_col[:], g_col_u8[:])

        feat_sb = persist.tile([BL, 2 * DIM], F32, tag="feat")
        oo_sb = persist.tile([BL, 2 * DIM], F32, tag="oo")

        # o tiles + expand-weight pools opened early so their space never
        # WAR-blocks on encoder pools (their DMAs are emitted later).
        o_pool = stk.enter_context(tc.tile_pool(name="o_pool", bufs=4))
        expw_pool = stk.enter_context(tc.tile_pool(name="expw", bufs=1))

        # ---------------- rgb encoder qkv (big; its loads lead the queues) -----------
        rgb_enc = _encoder_qkv(nc, tc, pst, persist, const, "rgb", 2048, rgb_d, wd,
                               ident, ones_row_bf)

        # ---------------- audio encoder qkv (tiny; overlaps rgb compute) -------------
        aud_enc = _encoder_qkv(nc, tc, pst, persist, const, "audio", 128, aud_d, wd,
                               ident, ones_row_bf)

        # attentions: audio first (LIFO pool close), both pipelined
        _attention(nc, tc, pst, const, "audio", wd, feat_sb, DIM, ones_row_bf,
                   ones128_bf, aud_enc)
        _attention(nc, tc, pst, const, "rgb", wd, feat_sb, 0, ones_row_bf,
                   ones128_bf, rgb_enc)

        # ---------------- o-mean (bf16 stream; 2-sample DMAs split Pool/SP) ----------
        om_ps = pst([BL, ODIM], tag="ps_om", bufs=1)
        o_view = o_d.rearrange("b (n p) d -> p b n d", p=128)  # [128, 16, 8, 512]
        PAIR = 2
        for c in range(BL // PAIR):
            o_sb = o_pool.tile([128, PAIR, O_T // 128, ODIM], BF16, tag="o_tile")
            eng = nc.gpsimd if c % 2 == 0 else nc.sync
            eng.dma_start(o_sb[:], o_view[:, PAIR * c : PAIR * (c + 1)])
            for bb in range(PAIR):
                b = PAIR * c + bb
                for j in range(O_T // 128):
                    nc.tensor.matmul(
                        om_ps[:],
                        sel16[:, b, :],
                        o_sb[:, bb, j, :],
                        start=(b == 0 and j == 0),
                        stop=(b == BL - 1 and j == O_T // 128 - 1),
                    )

        expw = expw_pool.tile([128, 4, 2 * DIM], BF16, tag="expw")
        nc.gpsimd.dma_start(expw[:], wd["expand_W"].rearrange("(c p) d -> p c d", p=128))
        expb = expw_pool.tile([1, 2 * DIM], BF16, tag="expb")
        nc.gpsimd.dma_start(expb[:], wd["expand_b"][None, :])

        om_sb = persist.tile([BL, ODIM], F32, tag="om")
        nc.scalar.activation(om_sb[:], om_ps[:], AF.Copy, scale=1.0 / O_T)
        omT = persist.tile([128, 4, BL], BF16, tag="omT")
        for c in range(4):
            tp = pst([128, BL])
            nc.tensor.transpose(tp[:], om_sb[:, 128 * c : 128 * (c + 1)], ident[:BL, :BL])
            nc.scalar.copy(omT[:, c, :], tp[:])

        # ---------------- expand + normalize -> oo ----------------
        if True:
            oo_ps = []
            for half in range(2):
                pp = pst([BL, DIM])
                for c in range(4):
                    nc.tensor.matmul(pp[:], omT[:, c, :], expw[:, c, 512 * half : 512 * (half + 1)],
                                     start=(c == 0), stop=False)
                nc.tensor.matmul(pp[:], ones_row_bf[:, :BL], expb[:, 512 * half : 512 * (half + 1)],
                                 start=False, stop=True)
                oo_ps.append(pp)
            sq_junk = persist.tile([BL, DIM], F32, tag="sq_junk")
            ss = [persist.tile([BL, 1], F32, tag=f"ss{i}", name=f"ss{i}") for i in range(2)]
            for half in range(2):
                nc.scalar.activation(sq_junk[:], oo_ps[half][:], AF.Square, accum_out=ss[half][:])
            nrm = persist.tile([BL, 1], F32, tag="nrm")
            nc.vector.tensor_tensor(nrm[:], ss[0][:], ss[1][:], AluOpType.add)
            nc.scalar.sqrt(nrm[:], nrm[:])
            nc.vector.tensor_scalar_max(nrm[:], nrm[:], 1e-12)
            rnrm = persist.tile([BL, 1], F32, tag="rnrm")
            nc.vector.reciprocal(rnrm[:], nrm[:])
            for half in range(2):
                nc.vector.tensor_scalar_mul(oo_sb[:, 512 * half : 512 * (half + 1)],
                                            oo_ps[half][:], rnrm[:])

        if stage == "enc":
            nc.sync.dma_start(dbg_d[0:BL, 0 : 2 * DIM], feat_sb[:])
            return

        if stage == "oenc":
            nc.sync.dma_start(dbg_d[0:BL, 0 : 2 * DIM], feat_sb[:])
            nc.sync.dma_start(dbg_d[0:BL, 2 * DIM :], oo_sb[:])
            return

        # ---------------- AllGather (split: oo early, feat late) ----------------
        import os
        ktime = bool(os.environ.get("KTIME"))
        ag_in_oo = dram.tile([BL, 2 * DIM], F32, tag="ag_in_oo")
        ag_out_oo = dram.tile([B, 2 * DIM], F32, tag="ag_out_oo")
        ag_in_ft = dram.tile([BL, 2 * DIM], F32, tag="ag_in_ft")
        ag_out_ft = dram.tile([B, 2 * DIM], F32, tag="ag_out_ft")
        nc.sync.dma_start(ag_in_oo[:], oo_sb[:])
        nc.sync.dma_start(ag_in_ft[:], feat_sb[:])

        def _ag(src, dst):
            if ktime:
                # collective-free stand-in for TimelineSim (cost model can't
                # model collectives); timing-equivalent except the AllGather.
                nc.sync.dma_start(dst[0:BL, :], src[:])
            else:
                nc.gpsimd.collective_compute(
                    "AllGather",
                    AluOpType.bypass,
                    replica_groups=[list(range(N_CORES))],
                    ins=[src.opt()],
                    outs=[dst.opt()],
                )

        _ag(ag_in_oo, ag_out_oo)
        _ag(ag_in_ft, ag_out_ft)

        # ---------------- ranking ----------------
        with tc.tile_pool(name="rank", bufs=1) as rank_pool:
            emb = rank_pool.tile([B, 4 * DIM], F32, tag="emb")
            nc.sync.dma_start(emb[:, 2 * DIM :], ag_out_oo[:])
            nc.scalar.dma_start(emb[:, : 2 * DIM], ag_out_ft[:])

            if stage == "ag":
                nc.sync.dma_start(dbg_d[:], emb[:])
                return

            # transpose emb -> embT [128, 16, 128]; chunks 0..7 featT, 8..15 ooT
            # (oo chunks 8..15 first — their gather lands much earlier)
            embT = rank_pool.tile([128, 16, 128], F32, tag="embT")
            for grp4 in (2, 3, 0, 1):
                tp = pst([128, 512])
                for j in range(4):
                    c = 4 * grp4 + j
                    nc.tensor.transpose(tp[:, 128 * j : 128 * (j + 1)],
                                        emb[:, 128 * c : 128 * (c + 1)], ident[:])
                nc.scalar.copy(embT[:, 4 * grp4 : 4 * grp4 + 4, :],
                               tp[:].rearrange("p (j c) -> p j c", j=4))

            G_ps = pst([B, B])
            for c in range(8):
                nc.tensor.matmul(G_ps[:], embT[:, 8 + c, :], embT[:, c, :],
                                 start=(c == 0), stop=(c == 7))
            G_sb = rank_pool.tile([B, B], F32, tag="G_sb")
            nc.scalar.copy(G_sb[:], G_ps[:])

            if stage == "rank1":
                nc.sync.dma_start(dbg_d[:, 0:B], G_sb[:])
                return

            junk = rank_pool.tile([B, B], F32, tag="junk")
            diag = rank_pool.tile([B, 1], F32, tag="diag")
            nc.vector.tensor_tensor(junk[:], G_sb[:], ident[:], AluOpType.mult)
            nc.vector.reduce_sum(diag[:], junk[:], axis=AX.X)
            mdiag = rank_pool.tile([B, 1], F32, tag="mdiag")
            nc.vector.tensor_scalar(mdiag[:], diag[:], -1.0, MARGIN,
                                    AluOpType.mult, AluOpType.add)

            Gt_ps = pst([B, B])
            nc.tensor.transpose(Gt_ps[:], G_sb[:], ident[:])
            Gt_sb = rank_pool.tile([B, B], F32, tag="Gt_sb")
            nc.scalar.copy(Gt_sb[:], Gt_ps[:])

            if stage == "rank1b":
                nc.sync.dma_start(dbg_d[:, 0:B], Gt_sb[:])
                nc.sync.dma_start(dbg_d[:, B : B + 1], diag[:])
                return

            # broadcast g along partitions: gb[m, n] = g[n], via colsums of a
            # zero-padded one-row matrix (K=1 matmuls are avoided).
            g_pad = rank_pool.tile([B, B], F32, tag="g_pad")
            nc.vector.memset(g_pad[:], 0.0)
            nc.vector.tensor_copy(g_pad[0:1, :], g_row[:])
            gb_ps = pst([B, B])
            nc.tensor.matmul(gb_ps[:], ones128[:], g_pad[:], start=True, stop=True)
            gneg_sb = rank_pool.tile([B, B], F32, tag="gneg_sb")
            nc.vector.tensor_scalar(gneg_sb[:], gb_ps[:], 1e30, -1e30,
                                    AluOpType.mult, AluOpType.add)

            stack = rank_pool.tile([B, 6], F32, tag="stack")
            Gm = rank_pool.tile([B, B], F32, tag="Gm")
            rmax = rank_pool.tile([B, 1], F32, tag="rmax")
            top = rank_pool.tile([B, 1], F32, tag="top")
            w = rank_pool.tile([B, 1], F32, tag="w")
            sel = rank_pool.tile([B, 1], F32, tag="sel")
            eq = rank_pool.tile([B, 1], F32, tag="eq")
            colv = rank_pool.tile([B, 1], F32, tag="colv")

            for di, (Gsrc, GsrcT) in enumerate(((G_sb, Gt_sb), (Gt_sb, G_sb))):
                T_sb = rank_pool.tile([B, B], F32, tag=f"T{di}")
                nc.scalar.activation(T_sb[:], Gsrc[:], AF.Relu, bias=mdiag[:])
                nc.vector.tensor_tensor(junk[:], T_sb[:], gb_ps[:], AluOpType.mult)
                nc.vector.reduce_sum(w[:], junk[:], axis=AX.X)
                nc.vector.tensor_tensor(stack[:, di : di + 1], w[:], g_col[:], AluOpType.mult)
                nc.vector.tensor_tensor(Gm[:], Gsrc[:], gneg_sb[:], AluOpType.add)
                nc.vector.reduce_max(rmax[:], Gm[:], axis=AX.X)
                nc.vector.tensor_tensor(top[:], diag[:], rmax[:], AluOpType.is_ge)
                # sel[i] = sum_b Gsrc[i,b]*g[b] as an N=1 matmul off GsrcT
                sel_ps = pst([B, 1])
                nc.tensor.matmul(sel_ps[:], GsrcT[:], g_col[:], start=True, stop=True)
                nc.vector.tensor_tensor(sel[:], sel_ps[:], g_col[:], AluOpType.mult)
                nc.vector.tensor_scalar(eq[:], sel[:], 0.0, None, AluOpType.is_equal)
                nc.vector.tensor_scalar(colv[:], eq[:], -1.0, 1.0,
                                        AluOpType.mult, AluOpType.add)
                nc.vector.tensor_copy(stack[:, 4 + di : 5 + di], colv[:])
                nc.vector.tensor_tensor(stack[:, 2 + di : 3 + di], colv[:], top[:],
                                        AluOpType.mult)

            if stage == "rank2":
                nc.sync.dma_start(dbg_d[:, 0:6], stack[:])
                nc.sync.dma_start(dbg_d[:, 8:136], Gt_sb[:])
                return

            # column-sums of the 6 stacked partials; the final scalar math
            # (divides by g-dependent counts) runs on host during unshard.
            S_ps = pst([1, 6])
            nc.tensor.matmul(S_ps[:], ones_col_f32[:], stack[:], start=True, stop=True)
            S_sb = rank_pool.tile([1, 6], F32, tag="S_sb")
            nc.vector.tensor_copy(S_sb[:], S_ps[:])

            if stage == "rank3":
                nc.sync.dma_start(dbg_d[0:1, 0:6], S_sb[:])
                return

            nc.sync.dma_start(out_d[:], S_sb[:])


def _encoder_qkv(nc, tc, pst, persist, const, mod, dm, x_d, wd,
                 ident, ones_row_bf):
    """Self-attention encoder, projection part: computes qT/kT/v_sb."""
    K = dm // 128
    n_tt = TOK // 128  # 8

    enc_pool_cm = tc.tile_pool(name=f"enc_{mod}", bufs=1)
    enc = enc_pool_cm.__enter__()
    qT = enc.tile([128, HEADS, TOK], BF16, tag="qT")
    kT = enc.tile([128, HEADS, TOK], BF16, tag="kT")
    v_sb = enc.tile([128, n_tt, DIM], BF16, tag="v_sb")
    poolT = enc.tile([128, HEADS, BL], BF16, tag="poolT")

    with ExitStack() as estk:
        xT_pool = estk.enter_context(tc.tile_pool(name=f"xT_{mod}", bufs=1))
        xT = xT_pool.tile([128, K, TOK], BF16, tag="xT")
        flat = x_d.rearrange("b t d -> (b t) d")
        # xT[d, tok] straight from DRAM via HWDGE xbar-transpose (bf16).
        # One transpose-DMA per 128-token row-block so downstream matmuls can
        # start after the first block lands; alternate SP/ACT queues.
        if K > 1:
            for tt in range(n_tt):
                eng = nc.sync if tt % 2 == 0 else nc.scalar
                eng.dma_start(xT[:, :, 128 * tt : 128 * (tt + 1)],
                              flat[128 * tt : 128 * (tt + 1), :], transpose=True)
        else:
            nc.sync.dma_start(xT[:, 0, :], flat[:, :], transpose=True)

        # v: lhsT = xT token-tile (stationary), rhs = Wv k-rows (moving)
        # audio's small loads ride the ACT HWDGE queue so they are not stuck
        # behind the o stream on Pool/SP.
        weng = nc.gpsimd if K > 1 else nc.scalar
        with tc.tile_pool(name=f"wv_{mod}", bufs=1) as wv_pool:
            wv = wv_pool.tile([128, K, DIM], BF16, tag="wv")
            wv_view = wd[f"{mod}_Wv"].rearrange("(kc p) d -> p kc d", p=128)
            # chunked so the first v matmuls unblock after ~1/4 of the load
            kchunk = max(K // 4, 1)
            for c0 in range(0, K, kchunk):
                weng.dma_start(wv[:, c0 : c0 + kchunk, :],
                               wv_view[:, c0 : c0 + kchunk, :])
            bv = wv_pool.tile([1, DIM], BF16, tag="bv")
            nc.gpsimd.dma_start(bv[:], wd[f"{mod}_bv"][None, :])
            for tt in range(n_tt):
                pv = pst([128, DIM])
                for kc in range(K):
                    nc.tensor.matmul(pv[:], xT[:, kc, 128 * tt : 128 * (tt + 1)], wv[:, kc, :],
                                     start=(kc == 0), stop=False)
                nc.tensor.matmul(pv[:], ones_row_bf[:], bv[:], start=False, stop=True)
                nc.vector.tensor_copy(v_sb[:, tt, :], pv[:])

        # q, k: lhsT = W column-block (stationary), rhs = xT (moving) -> [d, tok]
        bq_sb = const.tile([128, HEADS], F32, tag=f"bq_{mod}")
        nc.sync.dma_start(bq_sb[:], wd[f"{mod}_bq"].rearrange("(o p) -> p o", p=128))
        bk_sb = const.tile([128, HEADS], F32, tag=f"bk_{mod}")
        nc.sync.dma_start(bk_sb[:], wd[f"{mod}_bk"].rearrange("(o p) -> p o", p=128))
        with tc.tile_pool(name=f"wcol_{mod}", bufs=2) as wcol_pool:
            for pname, outT, b_sb in (("q", qT, bq_sb), ("k", kT, bk_sb)):
                w_d = wd[f"{mod}_W{pname}"].rearrange("(kc p) d -> p kc d", p=128)
                if K == 1:
                    wfull = wcol_pool.tile([128, DIM], BF16, tag="wfull", name="wfull")
                    nc.scalar.dma_start(wfull[:], w_d[:, 0, :])
                for dt_ in range(HEADS):
                    if K == 1:
                        wcol = wfull[:, None, 128 * dt_ : 128 * (dt_ + 1)]
                    else:
                        wcol = wcol_pool.tile([128, K, 128], BF16, tag="wcol",
                                              name="wcol")
                        nc.gpsimd.dma_start(wcol[:],
                                            w_d[:, :, 128 * dt_ : 128 * (dt_ + 1)])
                    for blk in range(TOK // 512):
                        pq = pst([128, 512])
                        for kc in range(K):
                            nc.tensor.matmul(pq[:], wcol[:, kc, :],
                                             xT[:, kc, 512 * blk : 512 * (blk + 1)],
                                             start=(kc == 0), stop=(kc == K - 1))
                        nc.scalar.activation(outT[:, dt_, 512 * blk : 512 * (blk + 1)], pq[:],
                                             AF.Identity, bias=b_sb[:, dt_ : dt_ + 1])

    return {"cm": enc_pool_cm, "qT": qT, "kT": kT, "v_sb": v_sb, "poolT": poolT}


def _attention(nc, tc, pst, const, mod, wd, feat_sb, feat_off, ones_row_bf,
               ones64_bf, enc):
    """Attention + pooling + out-proj; software-pipelined emission so the
    per-(grp,head) exp->sum->recip->mult chain overlaps across iterations.
    Writes feat_sb[:, feat_off:feat_off+512]."""
    qT, kT, v_sb, poolT = enc["qT"], enc["kT"], enc["v_sb"], enc["poolT"]
    scale = 1.0 / math.sqrt(HD)
    with ExitStack() as lstk:
        late = lstk.enter_context(tc.tile_pool(name=f"late_{mod}", bufs=1))
        avT = late.tile([128, HEADS, TOK], BF16, tag="avT")
        wo_pool = lstk.enter_context(tc.tile_pool(name=f"wo_{mod}", bufs=1))
        wo = wo_pool.tile([128, HEADS, DIM], BF16, tag="wo")
        nc.gpsimd.dma_start(wo[:], wd[f"{mod}_Wo"].rearrange("(h p) d -> p h d", p=128))
        w2 = wo_pool.tile([128, HEADS, DIM], BF16, tag="w2")
        nc.gpsimd.dma_start(w2[:], wd[f"{mod}_W2"].rearrange("(c p) d -> p c d", p=128))
        b2 = wo_pool.tile([1, DIM], BF16, tag="b2")
        nc.gpsimd.dma_start(b2[:], wd[f"{mod}_b2"][None, :])
        bo_sb = const.tile([128, HEADS], F32, tag=f"bo_{mod}")
        nc.sync.dma_start(bo_sb[:], wd[f"{mod}_bo"].rearrange("(o p) -> p o", p=128))
        ap = lstk.enter_context(tc.tile_pool(name=f"attn_{mod}", bufs=6))

        NIT = (BL // 8) * HEADS
        v8s, exps_t, sT8_t, rs_t, aT8_t = {}, {}, {}, {}, {}

        def stage_a(t):  # scores + exp
            grp, h = divmod(t, HEADS)
            if h == 0:
                v8 = ap.tile([64, 8, DIM], BF16, tag="v8")
                v8v = v8[:].rearrange("p (ul half) d -> p ul half d", half=2)
                nc.sync.dma_start(v8v[:, :, 0, :], v_sb[0:64, 4 * grp : 4 * grp + 4, :])
                nc.sync.dma_start(v8v[:, :, 1, :], v_sb[64:128, 4 * grp : 4 * grp + 4, :])
                v8s[grp] = v8
            sT8 = pst([64, 512])
            for i in range(8):
                b = 8 * grp + i
                nc.tensor.matmul(sT8[:, 64 * i : 64 * (i + 1)],
                                 kT[:, h, 64 * b : 64 * (b + 1)],
                                 qT[:, h, 64 * b : 64 * (b + 1)],
                                 start=True, stop=True)
            exps = ap.tile([64, 512], BF16, tag="exps")
            nc.scalar.activation(exps[:], sT8[:], AF.Exp, scale=scale)
            exps_t[t] = exps

        def stage_b(t):  # rowsum broadcast to 128 partitions + reciprocal
            rs = pst([128, 512])
            nc.tensor.matmul(rs[:], ones64_bf[0:64, :], exps_t[t][:],
                             start=True, stop=True)
            rrs = ap.tile([128, 512], F32, tag="rrs")
            nc.vector.reciprocal(rrs[:], rs[:])
            aT8_t[t] = rrs

        def stage_c(t):  # unnormalized attention-weighted v; divide fused in copy
            grp, h = divmod(t, HEADS)
            v8 = v8s[grp]
            avp = pst([128, 512])
            for i in range(8):
                nc.tensor.matmul(avp[:, 64 * i : 64 * (i + 1)],
                                 v8[:, i, 128 * h : 128 * (h + 1)],
                                 exps_t[t][:, 64 * i : 64 * (i + 1)],
                                 start=True, stop=True)
            nc.vector.tensor_tensor(avT[:, h, 512 * grp : 512 * (grp + 1)],
                                    avp[:], aT8_t[t][:], AluOpType.mult)

        pav_f = wo_pool.tile([128, HEADS, BL], F32, tag="pav_f")

        def stage_d(t):  # time-pool the finished (grp, h) slab
            grp, h = divmod(t, HEADS)
            nc.vector.reduce_sum(
                pav_f[:, h, 8 * grp : 8 * (grp + 1)],
                avT[:, h, 512 * grp : 512 * (grp + 1)].rearrange(
                    "p (s t) -> p s t", t=T),
                axis=AX.X)

        for t in range(NIT + 5):
            if t < NIT:
                stage_a(t)
            if 0 <= t - 2 < NIT:
                stage_b(t - 2)
            if 0 <= t - 4 < NIT:
                stage_c(t - 4)
            if 0 <= t - 5 < NIT:
                stage_d(t - 5)

        # out-proj (pooled) + W2
        _proj_w2(nc, tc, pst, wo_pool, mod, feat_sb, feat_off, pav_f, poolT,
                 ones_row_bf, wo, w2, b2, bo_sb)

    enc["cm"].__exit__(None, None, None)


def _proj_w2(nc, tc, pst, wo_pool, mod, feat_sb, feat_off, pav_f, poolT,
             ones_row_bf, wo, w2, b2, bo_sb):
    if True:
        # pool over time BEFORE the (linear) Wo projection: mean_t(av) @ Wo.
        # (pav_f filled by the attention pipeline's stage_d)
        pav = wo_pool.tile([128, HEADS, BL], BF16, tag="pav")
        nc.vector.tensor_copy(pav[:], pav_f[:])
        for dc in range(4):
            pT = pst([128, BL])
            for h in range(HEADS):
                nc.tensor.matmul(pT[:], wo[:, h, 128 * dc : 128 * (dc + 1)],
                                 pav[:, h, :], start=(h == 0), stop=(h == HEADS - 1))
            nc.scalar.activation(poolT[:, dc, :], pT[:], AF.Identity,
                                 bias=bo_sb[:, dc : dc + 1], scale=1.0 / T)

        pf = pst([BL, DIM])
        for c in range(HEADS):
            nc.tensor.matmul(pf[:], poolT[:, c, :], w2[:, c, :], start=(c == 0), stop=False)
        nc.tensor.matmul(pf[:], ones_row_bf[:, :BL], b2[:], start=False, stop=True)
        nc.scalar.copy(feat_sb[:, feat_off : feat_off + DIM], pf[:])


def kernel(**inputs):
    if "runner" not in _CACHE:
        _CACHE["runner"] = _make_runner()
    return _CACHE["runner"](inputs)


def _make_runner():
    nc = _build()
    import jax
    from jax.sharding import Mesh, PartitionSpec
    from jax.experimental.shard_map import shard_map
    from concourse import bass2jax

    bass2jax.install_neuronx_cc_hook()

    partition_name = nc.partition_id_tensor.name if nc.partition_id_tensor else None
    in_names, out_names, out_avals, zero_outs = [], [], [], []
    in_dtypes = {}
    for alloc in nc.m.functions[0].allocations:
        if not isinstance(alloc, mybir.MemoryLocationSet):
            continue
        name = alloc.memorylocations[0].name
        if alloc.kind == "ExternalInput":
            if name != partition_name:
                in_names.append(name)
                in_dtypes[name] = mybir.dt.np(alloc.dtype)
        elif alloc.kind == "ExternalOutput":
            out_names.append(name)
            shape = tuple(alloc.tensor_shape)
            dtype = mybir.dt.np(alloc.dtype)
            out_avals.append(jax.core.ShapedArray(shape, dtype))
            zero_outs.append(np.zeros(shape, dtype))
    n_params = len(in_names)
    all_in_names = list(in_names) + list(out_names)
    if partition_name is not None:
        all_in_names.append(partition_name)

    def _body(*args):
        operands = list(args)
        if partition_name is not None:
            operands.append(bass2jax.partition_id_tensor())
        outs = bass2jax._bass_exec_p.bind(
            *operands,
            out_avals=tuple(out_avals),
            in_names=tuple(all_in_names),
            out_names=tuple(out_names),
            lowering_input_output_aliases=(),
            sim_require_finite=True,
            sim_require_nnan=True,
            nc=nc,
        )
        return tuple(outs)

    devices = jax.devices()[:N_CORES]
    mesh = Mesh(np.asarray(devices), ("core",))
    in_specs = (PartitionSpec("core"),) * (n_params + len(out_names))
    out_specs = (PartitionSpec("core"),) * len(out_names)
    sharded = jax.jit(
        shard_map(_body, mesh=mesh, in_specs=in_specs, out_specs=out_specs,
                  check_rep=False),
        keep_unused=True,
    )

    out_idx = out_names.index("out")

    def run(inputs):
        per_core = _shard_inputs(inputs, in_dtypes)
        concat_in = [
            np.concatenate([per_core[c][name] for c in range(N_CORES)], axis=0)
            for name in in_names
        ]
        concat_zeros = [
            np.zeros((N_CORES * z.shape[0], *z.shape[1:]), z.dtype) for z in zero_outs
        ]
        out_arrs = sharded(*concat_in, *concat_zeros)
        run.last_outputs = {n: np.asarray(out_arrs[i]) for i, n in enumerate(out_names)}
        out = run.last_outputs["out"]  # [8, 6] column-sum partials, identical rows
        S = np.asarray(out[0], np.float64)
        sg = float(np.asarray(inputs["group_mask"]).astype(bool).sum())
        num = S[0] + S[1] - 2.0 * MARGIN * sg
        d1 = max(sg - 1.0, 1.0)
        d2 = max(min(max(sg - 1.0, 0.0), 1.0) * sg, 1.0)
        loss = num / d1 / d2
        acc = 0.5 * (S[2] / max(S[4], 1.0) + S[3] / max(S[5], 1.0))
        return np.float32(loss), np.float32(acc)

    run.sharded = sharded
    run.in_names = in_names
    run.in_dtypes = in_dtypes
    run.zero_outs = zero_outs
    run.nc = nc
    return run


def _shard_inputs(inputs, in_dtypes=None):
    if in_dtypes is None:
        in_dtypes = _CACHE["runner"].in_dtypes
    per_core = []
    gm = np.ascontiguousarray(np.asarray(inputs["group_mask"]).astype(np.uint8))
    shared = {}
    for k, v in inputs.items():
        if k not in ("o", "rgb", "audio", "group_mask"):
            shared[k] = np.ascontiguousarray(
                np.asarray(v).astype(in_dtypes.get(k, np.float32)))
    o = np.asarray(inputs["o"]).astype(in_dtypes.get("o", np.float32))
    rgb = np.asarray(inputs["rgb"]).astype(in_dtypes.get("rgb", np.float32))
    audio = np.asarray(inputs["audio"]).astype(in_dtypes.get("audio", np.float32))
    for c in range(N_CORES):
        sl = slice(BL * c, BL * (c + 1))
        m = {
            "o": np.ascontiguousarray(o[sl]),
            "rgb": np.ascontiguousarray(rgb[sl]),
            "audio": np.ascontiguousarray(audio[sl]),
            "group_mask": gm,
        }
        m.update(shared)
        per_core.append(m)
    return per_core


# revision 56
# speedup vs baseline: 1.2574x; 1.2574x over previous
"""Trainium2 Bass kernel for nn_CollaborativeExpertsWrapper.

Self-contained: shards batch B=128 across 8 NeuronCores (data-parallel
encoders), all-gathers [16, 2048] embeddings, each core redundantly computes
the masked ranking loss; host takes core 0's (loss, acc).

v2: full-bf16 datapath — inputs and weights are cast to bf16 on the host
(halves HBM traffic), xT is produced by HWDGE transpose-DMA (removes the
PE-transpose + ACT-copy pipeline), all projection matmuls run in bf16.
Accumulation stays fp32 in PSUM; the ranking block stays fp32.

v4 (TimelineSim 326us -> 223us): software-pipelined attention emission
(4 stages, lookahead 1/3/4); softmax denominator broadcast to 128
partitions so the divide fuses into the avT write (one DVE op saved per
head-group, aT8 tile gone); time-pooling moved BEFORE the Wo projection
(4 wide DVE reduces + 16 tiny matmuls replace 32 N=512 matmuls); o-mean
stream as 2-sample 2MB DMAs alternating Pool/SP queues; audio encoder's
small loads ride the ACT HWDGE queue so they are not stuck behind the o
stream; AllGather split in two (oo gathers ~60us earlier, hidden under
the attention drain); final mask-count divides moved to the host unshard
step (kernel outputs six partial sums).
"""
import sys

sys.path.insert(0, "/opt/trn_rl_repo")

import math
from contextlib import ExitStack

import numpy as np

import concourse.bacc as bacc
import concourse.bass as bass
import concourse.mybir as mybir
import concourse.tile as tile
from concourse.alu_op_type import AluOpType
from concourse.masks import make_identity

F32 = mybir.dt.float32
F32R = mybir.dt.float32r
BF16 = mybir.dt.bfloat16
U8 = mybir.dt.uint8
AF = mybir.ActivationFunctionType
AX = mybir.AxisListType

N_CORES = 8
B = 128
BL = B // N_CORES  # 16 samples per core
T = 64
DIM = 512
HEADS = 4
HD = DIM // HEADS  # 128
MARGIN = 1.0
TOK = BL * T  # 1024 tokens per core per modality
O_T = 1024
ODIM = 512

_CACHE = {}


def _build():
    nc = bacc.Bacc("TRN2", target_bir_lowering=False, debug=False, num_devices=N_CORES)

    o_d = nc.dram_tensor("o", [BL, O_T, ODIM], BF16, kind="ExternalInput").ap()
    rgb_d = nc.dram_tensor("rgb", [BL, T, 2048], BF16, kind="ExternalInput").ap()
    aud_d = nc.dram_tensor("audio", [BL, T, 128], BF16, kind="ExternalInput").ap()
    gm_d = nc.dram_tensor("group_mask", [B], U8, kind="ExternalInput").ap()

    wd = {}
    for m, dm in (("rgb", 2048), ("audio", 128)):
        for p in "qkv":
            wd[f"{m}_W{p}"] = nc.dram_tensor(f"{m}_W{p}", [dm, DIM], BF16, kind="ExternalInput").ap()
            wd[f"{m}_b{p}"] = nc.dram_tensor(f"{m}_b{p}", [DIM], F32, kind="ExternalInput").ap()
        wd[f"{m}_Wo"] = nc.dram_tensor(f"{m}_Wo", [DIM, DIM], BF16, kind="ExternalInput").ap()
        wd[f"{m}_bo"] = nc.dram_tensor(f"{m}_bo", [DIM], F32, kind="ExternalInput").ap()
        wd[f"{m}_W2"] = nc.dram_tensor(f"{m}_W2", [DIM, DIM], BF16, kind="ExternalInput").ap()
        wd[f"{m}_b2"] = nc.dram_tensor(f"{m}_b2", [DIM], F32, kind="ExternalInput").ap()
    wd["expand_W"] = nc.dram_tensor("expand_W", [DIM, 2 * DIM], BF16, kind="ExternalInput").ap()
    wd["expand_b"] = nc.dram_tensor("expand_b", [2 * DIM], F32, kind="ExternalInput").ap()

    out_d = nc.dram_tensor("out", [1, 6], F32, kind="ExternalOutput").ap()

    import os
    stage = os.environ.get("KSTAGE", "full")
    dbg_d = None
    if stage != "full":
        dbg_d = nc.dram_tensor("dbg", [B, 4 * DIM], F32, kind="ExternalOutput").ap()

    with tile.TileContext(nc) as tc:
        _emit(nc, tc, o_d, rgb_d, aud_d, gm_d, wd, out_d, stage, dbg_d)

    nc.compile()
    return nc


def _emit(nc, tc, o_d, rgb_d, aud_d, gm_d, wd, out_d, stage="full", dbg_d=None):
    stk = ExitStack()
    with stk:
        const = stk.enter_context(tc.tile_pool(name="const", bufs=1))
        persist = stk.enter_context(tc.tile_pool(name="persist", bufs=1))
        ps = stk.enter_context(tc.tile_pool(name="psum", bufs=7, space="PSUM"))
        dram = stk.enter_context(tc.tile_pool(name="dram", bufs=1, space="DRAM"))

        def pst(shape, tag="ps", bufs=None):
            return ps.tile(shape, F32, tag=tag, bufs=bufs, name=tag)

        # ---------------- constants ----------------
        ident = const.tile([128, 128], F32, tag="ident")
        make_identity(nc, ident)
        ones_col_f32 = const.tile([128, 1], F32, tag="ones_col_f32")
        nc.vector.memset(ones_col_f32[:], 1.0)
        ones64_s = const.tile([128, 128], F32, tag="ones64_s")
        nc.vector.memset(ones64_s[:], 0.0)
        nc.vector.memset(ones64_s[0:64, 0:64], 1.0)
        nc.vector.memset(ones64_s[64:128, 64:128], 1.0)
        ones_row_f32 = const.tile([1, 128], F32, tag="ones_row_f32")
        nc.vector.memset(ones_row_f32[:], 1.0)
        ones128 = const.tile([128, 128], F32, tag="ones128")
        nc.vector.memset(ones128[:], 1.0)
        ones_row_bf = const.tile([1, 128], BF16, tag="ones_row_bf")
        nc.vector.tensor_copy(ones_row_bf[:], ones_row_f32[:])
        sel16_s = const.tile([128, BL, BL], F32, tag="sel16_s")
        nc.vector.memset(sel16_s[:], 0.0)
        for b in range(BL):
            nc.vector.memset(sel16_s[:, b, b : b + 1], 1.0)
        sel16 = const.tile([128, BL, BL], BF16, tag="sel16")
        nc.vector.tensor_copy(sel16[:], sel16_s[:])
        ones128_bf = const.tile([128, 128], BF16, tag="ones128_bf")
        nc.vector.tensor_copy(ones128_bf[:], ones128[:])

        g_row_u8 = const.tile([1, B], U8, tag="g_row_u8")
        nc.sync.dma_start(g_row_u8[:], gm_d[None, :])
        g_row = const.tile([1, B], F32, tag="g_row")
        nc.vector.tensor_copy(g_row[:], g_row_u8[:])
        g_col_u8 = const.tile([B, 1], U8, tag="g_col_u8")
        nc.sync.dma_start(g_col_u8[:], gm_d[:, None])
        g_col = const.tile([B, 1], F32, tag="g_col")
        nc.vector.tensor_copy(g_col[:], g_col_u8[:])

        feat_sb = persist.tile([BL, 2 * DIM], F32, tag="feat")
        oo_sb = persist.tile([BL, 2 * DIM], F32, tag="oo")

        # o tiles + expand-weight pools opened early so their space never
        # WAR-blocks on encoder pools (their DMAs are emitted later).
        o_pool = stk.enter_context(tc.tile_pool(name="o_pool", bufs=4))
        expw_pool = stk.enter_context(tc.tile_pool(name="expw", bufs=1))

        # ---------------- rgb encoder qkv (big; its loads lead the queues) -----------
        rgb_enc = _encoder_qkv(nc, tc, pst, persist, const, "rgb", 2048, rgb_d, wd,
                               ident, ones_row_bf)

        # ---------------- audio encoder qkv (tiny; overlaps rgb compute) -------------
        aud_enc = _encoder_qkv(nc, tc, pst, persist, const, "audio", 128, aud_d, wd,
                               ident, ones_row_bf)

        # attentions: audio first (LIFO pool close), both pipelined
        _attention(nc, tc, pst, const, "audio", wd, feat_sb, DIM, ones_row_bf,
                   ones128_bf, aud_enc)
        _attention(nc, tc, pst, const, "rgb", wd, feat_sb, 0, ones_row_bf,
                   ones128_bf, rgb_enc)

        # ---------------- o-mean (bf16 stream; 2-sample DMAs split Pool/SP) ----------
        om_ps = pst([BL, ODIM], tag="ps_om", bufs=1)
        o_view = o_d.rearrange("b (n p) d -> p b n d", p=128)  # [128, 16, 8, 512]
        PAIR = 2
        for c in range(BL // PAIR):
            o_sb = o_pool.tile([128, PAIR, O_T // 128, ODIM], BF16, tag="o_tile")
            eng = nc.gpsimd if c % 2 == 0 else nc.sync
            eng.dma_start(o_sb[:], o_view[:, PAIR * c : PAIR * (c + 1)])
            for bb in range(PAIR):
                b = PAIR * c + bb
                for j in range(O_T // 128):
                    nc.tensor.matmul(
                        om_ps[:],
                        sel16[:, b, :],
                        o_sb[:, bb, j, :],
                        start=(b == 0 and j == 0),
                        stop=(b == BL - 1 and j == O_T // 128 - 1),
                    )

        expw = expw_pool.tile([128, 4, 2 * DIM], BF16, tag="expw")
        nc.gpsimd.dma_start(expw[:], wd["expand_W"].rearrange("(c p) d -> p c d", p=128))
        expb = expw_pool.tile([1, 2 * DIM], BF16, tag="expb")
        nc.gpsimd.dma_start(expb[:], wd["expand_b"][None, :])

        om_sb = persist.tile([BL, ODIM], F32, tag="om")
        nc.scalar.activation(om_sb[:], om_ps[:], AF.Copy, scale=1.0 / O_T)
        omT = persist.tile([128, 4, BL], BF16, tag="omT")
        for c in range(4):
            tp = pst([128, BL])
            nc.tensor.transpose(tp[:], om_sb[:, 128 * c : 128 * (c + 1)], ident[:BL, :BL])
            nc.scalar.copy(omT[:, c, :], tp[:])

        # ---------------- expand + normalize -> oo ----------------
        if True:
            oo_ps = []
            for half in range(2):
                pp = pst([BL, DIM])
                for c in range(4):
                    nc.tensor.matmul(pp[:], omT[:, c, :], expw[:, c, 512 * half : 512 * (half + 1)],
                                     start=(c == 0), stop=False)
                nc.tensor.matmul(pp[:], ones_row_bf[:, :BL], expb[:, 512 * half : 512 * (half + 1)],
                                 start=False, stop=True)
                oo_ps.append(pp)
            sq_junk = persist.tile([BL, DIM], F32, tag="sq_junk")
            ss = [persist.tile([BL, 1], F32, tag=f"ss{i}", name=f"ss{i}") for i in range(2)]
            for half in range(2):
                nc.scalar.activation(sq_junk[:], oo_ps[half][:], AF.Square, accum_out=ss[half][:])
            nrm = persist.tile([BL, 1], F32, tag="nrm")
            nc.vector.tensor_tensor(nrm[:], ss[0][:], ss[1][:], AluOpType.add)
            nc.scalar.sqrt(nrm[:], nrm[:])
            nc.vector.tensor_scalar_max(nrm[:], nrm[:], 1e-12)
            rnrm = persist.tile([BL, 1], F32, tag="rnrm")
            nc.vector.reciprocal(rnrm[:], nrm[:])
            for half in range(2):
                nc.vector.tensor_scalar_mul(oo_sb[:, 512 * half : 512 * (half + 1)],
                                            oo_ps[half][:], rnrm[:])

        if stage == "enc":
            nc.sync.dma_start(dbg_d[0:BL, 0 : 2 * DIM], feat_sb[:])
            return

        if stage == "oenc":
            nc.sync.dma_start(dbg_d[0:BL, 0 : 2 * DIM], feat_sb[:])
            nc.sync.dma_start(dbg_d[0:BL, 2 * DIM :], oo_sb[:])
            return

        # ---------------- AllGather (split: oo early, feat late) ----------------
        import os
        ktime = bool(os.environ.get("KTIME"))
        ag_in_oo = dram.tile([BL, 2 * DIM], F32, tag="ag_in_oo")
        ag_out_oo = dram.tile([B, 2 * DIM], F32, tag="ag_out_oo")
        ag_in_ft = dram.tile([BL, 2 * DIM], F32, tag="ag_in_ft")
        ag_out_ft = dram.tile([B, 2 * DIM], F32, tag="ag_out_ft")
        nc.sync.dma_start(ag_in_oo[:], oo_sb[:])
        nc.sync.dma_start(ag_in_ft[:], feat_sb[:])

        def _ag(src, dst):
            if ktime:
                # collective-free stand-in for TimelineSim (cost model can't
                # model collectives); timing-equivalent except the AllGather.
                nc.sync.dma_start(dst[0:BL, :], src[:])
            else:
                nc.gpsimd.collective_compute(
                    "AllGather",
                    AluOpType.bypass,
                    replica_groups=[list(range(N_CORES))],
                    ins=[src.opt()],
                    outs=[dst.opt()],
                )

        _ag(ag_in_oo, ag_out_oo)
        _ag(ag_in_ft, ag_out_ft)

        # ---------------- ranking ----------------
        with tc.tile_pool(name="rank", bufs=1) as rank_pool:
            emb = rank_pool.tile([B, 4 * DIM], F32, tag="emb")
            nc.sync.dma_start(emb[:, 2 * DIM :], ag_out_oo[:])
            nc.scalar.dma_start(emb[:, : 2 * DIM], ag_out_ft[:])

            if stage == "ag":
                nc.sync.dma_start(dbg_d[:], emb[:])
                return

            # transpose emb -> embT [128, 16, 128]; chunks 0..7 featT, 8..15 ooT
            # (oo chunks 8..15 first — their gather lands much earlier)
            embT = rank_pool.tile([128, 16, 128], F32, tag="embT")
            for grp4 in (2, 3, 0, 1):
                tp = pst([128, 512])
                for j in range(4):
                    c = 4 * grp4 + j
                    nc.tensor.transpose(tp[:, 128 * j : 128 * (j + 1)],
                                        emb[:, 128 * c : 128 * (c + 1)], ident[:])
                nc.scalar.copy(embT[:, 4 * grp4 : 4 * grp4 + 4, :],
                               tp[:].rearrange("p (j c) -> p j c", j=4))

            G_ps = pst([B, B])
            for c in range(8):
                nc.tensor.matmul(G_ps[:], embT[:, 8 + c, :], embT[:, c, :],
                                 start=(c == 0), stop=(c == 7))
            G_sb = rank_pool.tile([B, B], F32, tag="G_sb")
            nc.scalar.copy(G_sb[:], G_ps[:])

            if stage == "rank1":
                nc.sync.dma_start(dbg_d[:, 0:B], G_sb[:])
                return

            junk = rank_pool.tile([B, B], F32, tag="junk")
            diag = rank_pool.tile([B, 1], F32, tag="diag")
            nc.vector.tensor_tensor(junk[:], G_sb[:], ident[:], AluOpType.mult)
            nc.vector.reduce_sum(diag[:], junk[:], axis=AX.X)
            mdiag = rank_pool.tile([B, 1], F32, tag="mdiag")
            nc.vector.tensor_scalar(mdiag[:], diag[:], -1.0, MARGIN,
                                    AluOpType.mult, AluOpType.add)

            Gt_ps = pst([B, B])
            nc.tensor.transpose(Gt_ps[:], G_sb[:], ident[:])
            Gt_sb = rank_pool.tile([B, B], F32, tag="Gt_sb")
            nc.scalar.copy(Gt_sb[:], Gt_ps[:])

            if stage == "rank1b":
                nc.sync.dma_start(dbg_d[:, 0:B], Gt_sb[:])
                nc.sync.dma_start(dbg_d[:, B : B + 1], diag[:])
                return

            # broadcast g along partitions: gb[m, n] = g[n], via colsums of a
            # zero-padded one-row matrix (K=1 matmuls are avoided).
            g_pad = rank_pool.tile([B, B], F32, tag="g_pad")
            nc.vector.memset(g_pad[:], 0.0)
            nc.vector.tensor_copy(g_pad[0:1, :], g_row[:])
            gb_ps = pst([B, B])
            nc.tensor.matmul(gb_ps[:], ones128[:], g_pad[:], start=True, stop=True)
            gneg_sb = rank_pool.tile([B, B], F32, tag="gneg_sb")
            nc.vector.tensor_scalar(gneg_sb[:], gb_ps[:], 1e30, -1e30,
                                    AluOpType.mult, AluOpType.add)

            stack = rank_pool.tile([B, 6], F32, tag="stack")
            Gm = rank_pool.tile([B, B], F32, tag="Gm")
            rmax = rank_pool.tile([B, 1], F32, tag="rmax")
            top = rank_pool.tile([B, 1], F32, tag="top")
            w = rank_pool.tile([B, 1], F32, tag="w")
            sel = rank_pool.tile([B, 1], F32, tag="sel")
            eq = rank_pool.tile([B, 1], F32, tag="eq")
            colv = rank_pool.tile([B, 1], F32, tag="colv")

            for di, (Gsrc, GsrcT) in enumerate(((G_sb, Gt_sb), (Gt_sb, G_sb))):
                T_sb = rank_pool.tile([B, B], F32, tag=f"T{di}")
                nc.scalar.activation(T_sb[:], Gsrc[:], AF.Relu, bias=mdiag[:])
                nc.vector.tensor_tensor(junk[:], T_sb[:], gb_ps[:], AluOpType.mult)
                nc.vector.reduce_sum(w[:], junk[:], axis=AX.X)
                nc.vector.tensor_tensor(stack[:, di : di + 1], w[:], g_col[:], AluOpType.mult)
                nc.vector.tensor_tensor(Gm[:], Gsrc[:], gneg_sb[:], AluOpType.add)
                nc.vector.reduce_max(rmax[:], Gm[:], axis=AX.X)
                nc.vector.tensor_tensor(top[:], diag[:], rmax[:], AluOpType.is_ge)
                # sel[i] = sum_b Gsrc[i,b]*g[b] as an N=1 matmul off GsrcT
                sel_ps = pst([B, 1])
                nc.tensor.matmul(sel_ps[:], GsrcT[:], g_col[:], start=True, stop=True)
                nc.vector.tensor_tensor(sel[:], sel_ps[:], g_col[:], AluOpType.mult)
                nc.vector.tensor_scalar(eq[:], sel[:], 0.0, None, AluOpType.is_equal)
                nc.vector.tensor_scalar(colv[:], eq[:], -1.0, 1.0,
                                        AluOpType.mult, AluOpType.add)
                nc.vector.tensor_copy(stack[:, 4 + di : 5 + di], colv[:])
                nc.vector.tensor_tensor(stack[:, 2 + di : 3 + di], colv[:], top[:],
                                        AluOpType.mult)

            if stage == "rank2":
                nc.sync.dma_start(dbg_d[:, 0:6], stack[:])
                nc.sync.dma_start(dbg_d[:, 8:136], Gt_sb[:])
                return

            # column-sums of the 6 stacked partials; the final scalar math
            # (divides by g-dependent counts) runs on host during unshard.
            S_ps = pst([1, 6])
            nc.tensor.matmul(S_ps[:], ones_col_f32[:], stack[:], start=True, stop=True)
            S_sb = rank_pool.tile([1, 6], F32, tag="S_sb")
            nc.vector.tensor_copy(S_sb[:], S_ps[:])

            if stage == "rank3":
                nc.sync.dma_start(dbg_d[0:1, 0:6], S_sb[:])
                return

            nc.sync.dma_start(out_d[:], S_sb[:])


def _encoder_qkv(nc, tc, pst, persist, const, mod, dm, x_d, wd,
                 ident, ones_row_bf):
    """Self-attention encoder, projection part: computes qT/kT/v_sb."""
    K = dm // 128
    n_tt = TOK // 128  # 8

    enc_pool_cm = tc.tile_pool(name=f"enc_{mod}", bufs=1)
    enc = enc_pool_cm.__enter__()
    qT = enc.tile([128, HEADS, TOK], BF16, tag="qT")
    kT = enc.tile([128, HEADS, TOK], BF16, tag="kT")
    v_sb = enc.tile([128, n_tt, DIM], BF16, tag="v_sb")
    poolT = enc.tile([128, HEADS, BL], BF16, tag="poolT")

    with ExitStack() as estk:
        xT_pool = estk.enter_context(tc.tile_pool(name=f"xT_{mod}", bufs=1))
        xT = xT_pool.tile([128, K, TOK], BF16, tag="xT")
        flat = x_d.rearrange("b t d -> (b t) d")
        # xT[d, tok] straight from DRAM via HWDGE xbar-transpose (bf16).
        # One transpose-DMA per 128-token row-block so downstream matmuls can
        # start after the first block lands; alternate SP/ACT queues.
        if K > 1:
            for tt in range(n_tt):
                eng = nc.sync if tt % 2 == 0 else nc.scalar
                eng.dma_start(xT[:, :, 128 * tt : 128 * (tt + 1)],
                              flat[128 * tt : 128 * (tt + 1), :], transpose=True)
        else:
            nc.sync.dma_start(xT[:, 0, :], flat[:, :], transpose=True)

        # v: lhsT = xT token-tile (stationary), rhs = Wv k-rows (moving)
        # audio's small loads ride the ACT HWDGE queue so they are not stuck
        # behind the o stream on Pool/SP.
        weng = nc.gpsimd if K > 1 else nc.scalar
        with tc.tile_pool(name=f"wv_{mod}", bufs=1) as wv_pool:
            wv = wv_pool.tile([128, K, DIM], BF16, tag="wv")
            wv_view = wd[f"{mod}_Wv"].rearrange("(kc p) d -> p kc d", p=128)
            # chunked so the first v matmuls unblock after ~1/4 of the load
            kchunk = max(K // 4, 1)
            for c0 in range(0, K, kchunk):
                weng.dma_start(wv[:, c0 : c0 + kchunk, :],
                               wv_view[:, c0 : c0 + kchunk, :])
            bv = wv_pool.tile([1, DIM], BF16, tag="bv")
            nc.gpsimd.dma_start(bv[:], wd[f"{mod}_bv"][None, :])
            for tt in range(n_tt):
                pv = pst([128, DIM])
                for kc in range(K):
                    nc.tensor.matmul(pv[:], xT[:, kc, 128 * tt : 128 * (tt + 1)], wv[:, kc, :],
                                     start=(kc == 0), stop=False)
                nc.tensor.matmul(pv[:], ones_row_bf[:], bv[:], start=False, stop=True)
                nc.vector.tensor_copy(v_sb[:, tt, :], pv[:])

        # q, k: lhsT = W column-block (stationary), rhs = xT (moving) -> [d, tok]
        bq_sb = const.tile([128, HEADS], F32, tag=f"bq_{mod}")
        nc.sync.dma_start(bq_sb[:], wd[f"{mod}_bq"].rearrange("(o p) -> p o", p=128))
        bk_sb = const.tile([128, HEADS], F32, tag=f"bk_{mod}")
        nc.sync.dma_start(bk_sb[:], wd[f"{mod}_bk"].rearrange("(o p) -> p o", p=128))
        with tc.tile_pool(name=f"wcol_{mod}", bufs=2) as wcol_pool:
            for pname, outT, b_sb in (("q", qT, bq_sb), ("k", kT, bk_sb)):
                w_d = wd[f"{mod}_W{pname}"].rearrange("(kc p) d -> p kc d", p=128)
                if K == 1:
                    wfull = wcol_pool.tile([128, DIM], BF16, tag="wfull", name="wfull")
                    nc.scalar.dma_start(wfull[:], w_d[:, 0, :])
                for dt_ in range(HEADS):
                    if K == 1:
                        wcol = wfull[:, None, 128 * dt_ : 128 * (dt_ + 1)]
                    else:
                        wcol = wcol_pool.tile([128, K, 128], BF16, tag="wcol",
                                              name="wcol")
                        nc.gpsimd.dma_start(wcol[:],
                                            w_d[:, :, 128 * dt_ : 128 * (dt_ + 1)])
                    for blk in range(TOK // 512):
                        pq = pst([128, 512])
                        for kc in range(K):
                            nc.tensor.matmul(pq[:], wcol[:, kc, :],
                                             xT[:, kc, 512 * blk : 512 * (blk + 1)],
                                             start=(kc == 0), stop=(kc == K - 1))
                        nc.scalar.activation(outT[:, dt_, 512 * blk : 512 * (blk + 1)], pq[:],
                                             AF.Identity, bias=b_sb[:, dt_ : dt_ + 1])

    return {"cm": enc_pool_cm, "qT": qT, "kT": kT, "v_sb": v_sb, "poolT": poolT}


def _attention(nc, tc, pst, const, mod, wd, feat_sb, feat_off, ones_row_bf,
               ones64_bf, enc):
    """Attention + pooling + out-proj; software-pipelined emission so the
    per-(grp,head) exp->sum->recip->mult chain overlaps across iterations.
    Writes feat_sb[:, feat_off:feat_off+512]."""
    qT, kT, v_sb, poolT = enc["qT"], enc["kT"], enc["v_sb"], enc["poolT"]
    scale = 1.0 / math.sqrt(HD)
    with ExitStack() as lstk:
        late = lstk.enter_context(tc.tile_pool(name=f"late_{mod}", bufs=1))
        avT = late.tile([128, HEADS, TOK], BF16, tag="avT")
        wo_pool = lstk.enter_context(tc.tile_pool(name=f"wo_{mod}", bufs=1))
        wo = wo_pool.tile([128, HEADS, DIM], BF16, tag="wo")
        nc.gpsimd.dma_start(wo[:], wd[f"{mod}_Wo"].rearrange("(h p) d -> p h d", p=128))
        w2 = wo_pool.tile([128, HEADS, DIM], BF16, tag="w2")
        nc.gpsimd.dma_start(w2[:], wd[f"{mod}_W2"].rearrange("(c p) d -> p c d", p=128))
        b2 = wo_pool.tile([1, DIM], BF16, tag="b2")
        nc.gpsimd.dma_start(b2[:], wd[f"{mod}_b2"][None, :])
        bo_sb = const.tile([128, HEADS], F32, tag=f"bo_{mod}")
        nc.sync.dma_start(bo_sb[:], wd[f"{mod}_bo"].rearrange("(o p) -> p o", p=128))
        ap = lstk.enter_context(tc.tile_pool(name=f"attn_{mod}", bufs=4))

        NIT = (BL // 8) * HEADS
        v8s, exps_t, sT8_t, rs_t, aT8_t = {}, {}, {}, {}, {}

        def stage_a(t):  # scores + exp
            grp, h = divmod(t, HEADS)
            if h == 0:
                v8 = ap.tile([64, 8, DIM], BF16, tag="v8")
                v8v = v8[:].rearrange("p (ul half) d -> p ul half d", half=2)
                nc.sync.dma_start(v8v[:, :, 0, :], v_sb[0:64, 4 * grp : 4 * grp + 4, :])
                nc.sync.dma_start(v8v[:, :, 1, :], v_sb[64:128, 4 * grp : 4 * grp + 4, :])
                v8s[grp] = v8
            sT8 = pst([64, 512])
            for i in range(8):
                b = 8 * grp + i
                nc.tensor.matmul(sT8[:, 64 * i : 64 * (i + 1)],
                                 kT[:, h, 64 * b : 64 * (b + 1)],
                                 qT[:, h, 64 * b : 64 * (b + 1)],
                                 start=True, stop=True)
            exps = ap.tile([64, 512], BF16, tag="exps")
            nc.scalar.activation(exps[:], sT8[:], AF.Exp, scale=scale)
            exps_t[t] = exps

        def stage_b(t):  # rowsum broadcast to 128 partitions + reciprocal
            rs = pst([128, 512])
            nc.tensor.matmul(rs[:], ones64_bf[0:64, :], exps_t[t][:],
                             start=True, stop=True)
            rrs = ap.tile([128, 512], F32, tag="rrs")
            nc.vector.reciprocal(rrs[:], rs[:])
            aT8_t[t] = rrs

        def stage_c(t):  # unnormalized attention-weighted v; divide fused in copy
            grp, h = divmod(t, HEADS)
            v8 = v8s[grp]
            avp = pst([128, 512])
            for i in range(8):
                nc.tensor.matmul(avp[:, 64 * i : 64 * (i + 1)],
                                 v8[:, i, 128 * h : 128 * (h + 1)],
                                 exps_t[t][:, 64 * i : 64 * (i + 1)],
                                 start=True, stop=True)
            nc.vector.tensor_tensor(avT[:, h, 512 * grp : 512 * (grp + 1)],
                                    avp[:], aT8_t[t][:], AluOpType.mult)

        pav_f = wo_pool.tile([128, HEADS, BL], F32, tag="pav_f")

        def stage_d(t):  # time-pool the finished (grp, h) slab
            grp, h = divmod(t, HEADS)
            nc.vector.reduce_sum(
                pav_f[:, h, 8 * grp : 8 * (grp + 1)],
                avT[:, h, 512 * grp : 512 * (grp + 1)].rearrange(
                    "p (s t) -> p s t", t=T),
                axis=AX.X)

        for t in range(NIT + 4):
            if t < NIT:
                stage_a(t)
            if 0 <= t - 1 < NIT:
                stage_b(t - 1)
            if 0 <= t - 3 < NIT:
                stage_c(t - 3)
            if 0 <= t - 4 < NIT:
                stage_d(t - 4)

        # out-proj (pooled) + W2
        _proj_w2(nc, tc, pst, wo_pool, mod, feat_sb, feat_off, pav_f, poolT,
                 ones_row_bf, wo, w2, b2, bo_sb)

    enc["cm"].__exit__(None, None, None)


def _proj_w2(nc, tc, pst, wo_pool, mod, feat_sb, feat_off, pav_f, poolT,
             ones_row_bf, wo, w2, b2, bo_sb):
    if True:
        # pool over time BEFORE the (linear) Wo projection: mean_t(av) @ Wo.
        # (pav_f filled by the attention pipeline's stage_d)
        pav = wo_pool.tile([128, HEADS, BL], BF16, tag="pav")
        nc.vector.tensor_copy(pav[:], pav_f[:])
        for dc in range(4):
            pT = pst([128, BL])
            for h in range(HEADS):
                nc.tensor.matmul(pT[:], wo[:, h, 128 * dc : 128 * (dc + 1)],
                                 pav[:, h, :], start=(h == 0), stop=(h == HEADS - 1))
            nc.scalar.activation(poolT[:, dc, :], pT[:], AF.Identity,
                                 bias=bo_sb[:, dc : dc + 1], scale=1.0 / T)

        pf = pst([BL, DIM])
        for c in range(HEADS):
            nc.tensor.matmul(pf[:], poolT[:, c, :], w2[:, c, :], start=(c == 0), stop=False)
        nc.tensor.matmul(pf[:], ones_row_bf[:, :BL], b2[:], start=False, stop=True)
        nc.scalar.copy(feat_sb[:, feat_off : feat_off + DIM], pf[:])


def kernel(**inputs):
    if "runner" not in _CACHE:
        _CACHE["runner"] = _make_runner()
    return _CACHE["runner"](inputs)


def _make_runner():
    nc = _build()
    import jax
    from jax.sharding import Mesh, PartitionSpec
    from jax.experimental.shard_map import shard_map
    from concourse import bass2jax

    bass2jax.install_neuronx_cc_hook()

    partition_name = nc.partition_id_tensor.name if nc.partition_id_tensor else None
    in_names, out_names, out_avals, zero_outs = [], [], [], []
    in_dtypes = {}
    for alloc in nc.m.functions[0].allocations:
        if not isinstance(alloc, mybir.MemoryLocationSet):
            continue
        name = alloc.memorylocations[0].name
        if alloc.kind == "ExternalInput":
            if name != partition_name:
                in_names.append(name)
                in_dtypes[name] = mybir.dt.np(alloc.dtype)
        elif alloc.kind == "ExternalOutput":
            out_names.append(name)
            shape = tuple(alloc.tensor_shape)
            dtype = mybir.dt.np(alloc.dtype)
            out_avals.append(jax.core.ShapedArray(shape, dtype))
            zero_outs.append(np.zeros(shape, dtype))
    n_params = len(in_names)
    all_in_names = list(in_names) + list(out_names)
    if partition_name is not None:
        all_in_names.append(partition_name)

    def _body(*args):
        operands = list(args)
        if partition_name is not None:
            operands.append(bass2jax.partition_id_tensor())
        outs = bass2jax._bass_exec_p.bind(
            *operands,
            out_avals=tuple(out_avals),
            in_names=tuple(all_in_names),
            out_names=tuple(out_names),
            lowering_input_output_aliases=(),
            sim_require_finite=True,
            sim_require_nnan=True,
            nc=nc,
        )
        return tuple(outs)

    devices = jax.devices()[:N_CORES]
    mesh = Mesh(np.asarray(devices), ("core",))
    in_specs = (PartitionSpec("core"),) * (n_params + len(out_names))
    out_specs = (PartitionSpec("core"),) * len(out_names)
    sharded = jax.jit(
        shard_map(_body, mesh=mesh, in_specs=in_specs, out_specs=out_specs,
                  check_rep=False),
        keep_unused=True,
    )

    out_idx = out_names.index("out")

    def run(inputs):
        per_core = _shard_inputs(inputs, in_dtypes)
        concat_in = [
            np.concatenate([per_core[c][name] for c in range(N_CORES)], axis=0)
            for name in in_names
        ]
        concat_zeros = [
            np.zeros((N_CORES * z.shape[0], *z.shape[1:]), z.dtype) for z in zero_outs
        ]
        out_arrs = sharded(*concat_in, *concat_zeros)
        run.last_outputs = {n: np.asarray(out_arrs[i]) for i, n in enumerate(out_names)}
        out = run.last_outputs["out"]  # [8, 6] column-sum partials, identical rows
        S = np.asarray(out[0], np.float64)
        sg = float(np.asarray(inputs["group_mask"]).astype(bool).sum())
        num = S[0] + S[1] - 2.0 * MARGIN * sg
        d1 = max(sg - 1.0, 1.0)
        d2 = max(min(max(sg - 1.0, 0.0), 1.0) * sg, 1.0)
        loss = num / d1 / d2
        acc = 0.5 * (S[2] / max(S[4], 1.0) + S[3] / max(S[5], 1.0))
        return np.float32(loss), np.float32(acc)

    run.sharded = sharded
    run.in_names = in_names
    run.in_dtypes = in_dtypes
    run.zero_outs = zero_outs
    run.nc = nc
    return run


def _shard_inputs(inputs, in_dtypes=None):
    if in_dtypes is None:
        in_dtypes = _CACHE["runner"].in_dtypes
    per_core = []
    gm = np.ascontiguousarray(np.asarray(inputs["group_mask"]).astype(np.uint8))
    shared = {}
    for k, v in inputs.items():
        if k not in ("o", "rgb", "audio", "group_mask"):
            shared[k] = np.ascontiguousarray(
                np.asarray(v).astype(in_dtypes.get(k, np.float32)))
    o = np.asarray(inputs["o"]).astype(in_dtypes.get("o", np.float32))
    rgb = np.asarray(inputs["rgb"]).astype(in_dtypes.get("rgb", np.float32))
    audio = np.asarray(inputs["audio"]).astype(in_dtypes.get("audio", np.float32))
    for c in range(N_CORES):
        sl = slice(BL * c, BL * (c + 1))
        m = {
            "o": np.ascontiguousarray(o[sl]),
            "rgb": np.ascontiguousarray(rgb[sl]),
            "audio": np.ascontiguousarray(audio[sl]),
            "group_mask": gm,
        }
        m.update(shared)
        per_core.append(m)
    return per_core
